# revision 29
# baseline (speedup 1.0000x reference)
"""GQA attention (B=2, N=2048, D=4096, 32 Q heads / 8 KV heads, rope, causal)
on 8 Trainium2 NeuronCores.

Strategy: tensor-parallel over KV heads (1 KV head + its 4 grouped Q heads per
core), transposed-flash attention without max-subtraction (scores are bounded,
verified ~[-10, 10]), AllToAll to convert the head-sharded attention output to
token-sharded, then each core runs the wo projection for its 512-token shard.
Host assembles the 8 token shards. All matmuls bf16 with fp32 accumulation.

Optimizations vs the first working version:
 - BIR post-passes: (1) dedupe back-to-back LDWEIGHTS with identical
   stationary APs (PE reuses the loaded weights; the two heads of a group
   share kT / ones / v stationaries), (2) thin out per-matmul semaphore
   increments within wait-free PE runs and rewrite all wait thresholds
   (walrus only accepts UpdateValue==1), since EVT_SEM writes serialize at
   ~26ns each on the engine.
 - Stage 2: scores for both grouped heads land in one 2-bank PSUM megatile
   and are exp'd by a single wide ACT instruction (ACT costs (N+352)/1.2 ns
   per instruction, so 512-col activations waste 40%); causal mask multiply
   on DVE (GpSimd SBUF access locks the shared DVE port); P is buffered
   LAG=4 kt tiles so score matmuls run ahead of ones/PV matmuls; each
   round's normalize/scatter chain is deferred behind the next round's
   first exp so the ACT FIFO never starves the PE; op PSUM banks are freed
   by a fast ACT copy.
 - DMA scheduling: engine FIFOs suffer head-of-line blocking on triggers
   with long semaphore waits, so a2a-output loads are emitted inside their
   stage-4 pass, v transposes go SBUF->SBUF on the sync queue, and weight /
   x loads are wide rearranged DMAs interleaved in consumption order on the
   scalar/sync queues (first matmul issues at ~15us instead of ~45us).
 - Stage 4: two pipelined passes (one per collective group); pass-A
   partials spill to DRAM bf16 and reload as batched prefetched DMAs.
 - NOTE: the chip power-throttles the PE to K=13/16 (~1.95 GHz) under
   sustained dense bf16 matmul; wall time is then bounded by streaming
   cycles (~869us) + gaps, and per-matmul overheads hide under the slower
   clock. fp8 variants all fail the 2e-2 max-rel-err gate (measured).

Layout notes:
 - All projections contract over the model dim, so both operands keep that dim
   on SBUF partitions: host passes xT [D, TOK] and transposed weight shards.
 - RoPE pairs are permuted so pair elements sit 64 partitions apart (even orig
   rows -> partitions 0..63, odd -> 64..127), making rope elementwise DVE ops
   on partition-halves. The same permutation applied to wq and wk rows leaves
   q.k dot products unchanged.
 - Scores are computed transposed, S_T[ktok, qtok], so PV needs no transpose of
   the probabilities. The softmax denominator accumulates on the PE as a
   bf16 ones-matmul into PSUM (every output row = the column sum), and the
   diagonal-band tiles restrict their matmul N to the unmasked column range so
   only one 128x128 triangle mask is ever applied.
"""

import sys

for _p in ("/opt/trn_rl_repo",):
    if _p not in sys.path:
        sys.path.append(_p)

import numpy as np
import ml_dtypes

BF16 = ml_dtypes.bfloat16
NC = 8
HD = 128
TB = 512  # token block (matmul moving size / psum bank)
KP = 128  # contraction chunk (partition size)

def _hgroups(HQ):
    """Head groups per collective: a big first group for early a2a overlap,
    then single-head groups so the last collective is small and the stage-4
    tail after it is short."""
    half = (HQ + 1) // 2
    return [g for g in (list(range(half)), list(range(half, HQ))) if g]


DEDUPE_LDW = True
COALESCE_UPDATES = True
COALESCE_CAP = 10


# --------------------------------------------------------------------------
# walrus workaround: TPB_CTRL-class instructions in this container accept only
# one semaphore wait; hoist excess waits onto preceding NoOps (same engine).
def _split_wide_waits(nc, mybir, maxw=1):
    ctr = 0
    for fn in nc.m.functions:
        for bb in fn.blocks:
            insts = bb.instructions
            newlist = []
            changed = False
            for inst in insts:
                si = inst.sync_info
                if si is not None and si.on_wait and len(si.on_wait) > maxw:
                    waits = list(si.on_wait)
                    k = 0
                    while len(waits) - k > maxw:
                        chunk = waits[k : k + maxw]
                        k += maxw
                        nop = mybir.InstNoOp(name=f"wsplit-{ctr}", ins=[], outs=[])
                        ctr += 1
                        nop.engine = inst.engine
                        nop.sync_info = mybir.SyncInfo(on_wait=chunk, on_update=[])
                        newlist.append(nop)
                        changed = True
                    si.on_wait = waits[k:]
                newlist.append(inst)
            if changed:
                insts.clear()
                insts.extend(newlist)


def _ap_key(ap):
    try:
        return (str(ap.concise()), ap.offset)
    except Exception:
        return object()  # unique -> never matches


def _dedupe_ldweights(nc, mybir):
    """Delete InstLdweights whose stationary AP matches the weights already
    resident in the PE array (loaded by the immediately preceding LDW) and
    which carry no semaphore waits. bf16-only kernels tolerate the
    standalone-LDW + non-self-loading-matmul pattern."""
    PE = None
    removed = 0
    for fn in nc.m.functions:
        for bb in fn.blocks:
            insts = bb.instructions
            newlist = []
            cur_key = None
            changed = False
            for inst in insts:
                tn = type(inst).__name__
                eng = getattr(inst, "engine", None)
                if eng is None or "PE" not in str(eng):
                    newlist.append(inst)
                    continue
                if tn == "InstLdweights":
                    si = inst.sync_info
                    has_wait = si is not None and bool(si.on_wait)
                    has_upd = si is not None and bool(si.on_update)
                    key = _ap_key(inst.ins[0])
                    if (
                        cur_key is not None
                        and key == cur_key
                        and not has_wait
                        and not has_upd
                    ):
                        removed += 1
                        changed = True
                        continue  # drop: weights already loaded
                    cur_key = key
                    newlist.append(inst)
                elif tn == "InstMatmult":
                    newlist.append(inst)
                elif tn in ("InstNoOp", "InstEventSemaphore"):
                    newlist.append(inst)
                else:
                    cur_key = None  # conservative reset
                    newlist.append(inst)
            if changed:
                insts.clear()
                insts.extend(newlist)
    return removed


def _coalesce_pe_updates(nc, mybir, cap=COALESCE_CAP):
    """Thin out per-matmul semaphore increments: within runs of wait-free PE
    instructions, drop all but the run's last increment of each semaphore,
    then rewrite every wait threshold on that semaphore to the new (smaller)
    cumulative counts. walrus only accepts UpdateValue==1, so counts shrink
    instead of summing. A wait originally satisfied by the t-th increment is
    remapped to the first KEPT increment at-or-after t (same run, so the
    delay is bounded by the run length and cannot deadlock)."""
    # ---- collect per-sem updater info across all blocks --------------
    sem_updates = {}  # sem_id -> list of (block, inst, update)
    sem_bad = set()
    for fn in nc.m.functions:
        for bb in fn.blocks:
            for inst in bb.instructions:
                si = inst.sync_info
                if si is None or not si.on_update:
                    continue
                eng = str(getattr(inst, "engine", ""))
                for u in si.on_update:
                    if str(getattr(u, "sync_type", "")) != "semaphore":
                        continue
                    ok = (
                        "PE" in eng
                        and type(inst).__name__ == "InstMatmult"
                        and getattr(u, "update_reg", None) is None
                        and getattr(u, "update_mode", None) == "sem-inc"
                        and getattr(u, "update_value", None) == 1
                    )
                    if ok:
                        sem_updates.setdefault(u.id, []).append((bb, inst, u))
                    else:
                        sem_bad.add(u.id)
    # candidate sems: all updates are simple PE matmul incs, single block
    cand = {}
    for sid, lst in sem_updates.items():
        if sid in sem_bad:
            continue
        blocks = {id(bb) for bb, _, _ in lst}
        if len(blocks) != 1 or len(lst) < 4:
            continue
        cand[sid] = lst
    if not cand:
        return 0
    # waits must all be plain ge-imm
    sem_waits = {}  # sem_id -> list of wait objects
    for fn in nc.m.functions:
        for bb in fn.blocks:
            for inst in bb.instructions:
                si = inst.sync_info
                if si is None or not si.on_wait:
                    continue
                for w in si.on_wait:
                    if str(getattr(w, "sync_type", "")) != "semaphore":
                        continue
                    if w.id in cand:
                        if (
                            getattr(w, "wait_reg", None) is not None
                            or getattr(w, "wait_mode", None) != "sem-ge-imm"
                        ):
                            cand.pop(w.id, None)
                        else:
                            sem_waits.setdefault(w.id, []).append(w)
    if not cand:
        return 0
    # ---- build runs over each candidate block's PE sequence ----------
    dropped = 0
    for sid, lst in cand.items():
        bb = lst[0][0]
        # PE instruction sequence of this block
        keep = {}  # id(update) -> bool
        inc_seq = []  # updates in PE program order
        run_updates = []  # updates in current run

        def flush_run():
            # keep only the last update (of this sem) in the run
            for u in run_updates[:-1]:
                keep[id(u)] = False
            if run_updates:
                keep[id(run_updates[-1])] = True
            run_updates.clear()

        for inst in bb.instructions:
            eng = str(getattr(inst, "engine", ""))
            if "PE" not in eng:
                continue
            tn = type(inst).__name__
            si = inst.sync_info
            has_wait = si is not None and bool(si.on_wait)
            if tn not in ("InstMatmult", "InstLdweights", "InstNoOp"):
                flush_run()
                continue
            if has_wait or len(run_updates) >= cap:
                flush_run()
            if si is not None and si.on_update:
                for u in si.on_update:
                    if (
                        str(getattr(u, "sync_type", "")) == "semaphore"
                        and u.id == sid
                    ):
                        inc_seq.append(u)
                        run_updates.append(u)
        flush_run()
        if not inc_seq:
            continue
        keep[id(inc_seq[-1])] = True  # final inc always kept
        # map original threshold t (1-based) -> new threshold
        kept_prefix = []
        k = 0
        for u in inc_seq:
            if keep.get(id(u), True):
                k += 1
            kept_prefix.append(k)
        T = len(inc_seq)
        # next kept index at-or-after t
        new_thresh = [0] * (T + 1)
        nk = T
        for t in range(T, 0, -1):
            if keep.get(id(inc_seq[t - 1]), True):
                nk = t
            new_thresh[t] = kept_prefix[nk - 1]
        # rewrite waits
        ok = True
        for w in sem_waits.get(sid, []):
            t = w.wait_value
            if not isinstance(t, int) or t < 1 or t > T:
                ok = False
                break
        if not ok:
            continue
        for w in sem_waits.get(sid, []):
            w.wait_value = new_thresh[w.wait_value]
        # physically remove dropped updates
        for inst in bb.instructions:
            si = inst.sync_info
            if si is None or not si.on_update:
                continue
            newu = [
                u
                for u in si.on_update
                if not (
                    str(getattr(u, "sync_type", "")) == "semaphore"
                    and u.id == sid
                    and keep.get(id(u), True) is False
                )
            ]
            if len(newu) != len(si.on_update):
                dropped += len(si.on_update) - len(newu)
                si.on_update = newu
    return dropped


def build_attention_nc(B, N, D, NH, NKV, split_waits=True):
    import concourse.bass as bass
    import concourse.mybir as mybir
    import concourse.tile as tile

    HQ = NH // NC  # q heads per core
    assert NKV == NC and NH // NKV == HQ
    DQ = NH * HD  # attention (q) total dims == wo contraction dim
    TOK = B * N
    NTB = TOK // TB  # token blocks (stage 1)
    NBB = N // TB  # token blocks per batch
    KC = D // KP  # contraction chunks for qkv proj
    KCQ = DQ // KP  # contraction chunks for wo proj
    MO = D // KP  # output-dim tiles for wo proj
    TSH = TOK // NC  # token shard per core (wo stage)
    SD = HQ * HD  # my attention dims (a2a shard rows)
    NKT = N // KP  # k tiles per batch
    HH = HD // 2
    F32 = mybir.dt.float32
    BF = mybir.dt.bfloat16
    AX = mybir.AluOpType
    AF = mybir.ActivationFunctionType
    SCALE = 1.0 / float(np.sqrt(HD))
    NXB = 8  # x contraction chunks per DMA trigger

    nc = bass.Bass("TRN2", num_devices=NC)
    xT = nc.declare_dram_parameter("xT", [D, TOK], BF, isOutput=False)
    wqT = nc.declare_dram_parameter("wqT", [D, SD], BF, isOutput=False)
    wkT = nc.declare_dram_parameter("wkT", [D, HD], BF, isOutput=False)
    wvT = nc.declare_dram_parameter("wvT", [D, HD], BF, isOutput=False)
    woL = nc.declare_dram_parameter("woL", [MO, KP, DQ], BF, isOutput=False)
    cosP = nc.declare_dram_parameter("cosP", [HD, N], F32, isOutput=False)
    sinP = nc.declare_dram_parameter("sinP", [HD, N], F32, isOutput=False)
    cmask = nc.declare_dram_parameter("cmask", [KP, KP], BF, isOutput=False)
    finalT = nc.declare_dram_parameter("finalT", [D, TSH], F32, isOutput=True)

    with tile.TileContext(nc) as tc:
        with (
            tc.tile_pool(name="dram", bufs=1, space="DRAM") as dram,
        ):
            hgroups = _hgroups(HQ)
            a2a_in = [
                dram.tile(
                    [NC * len(g) * HD, TSH], BF, tag=f"a2a_in{gi}", name=f"a2a_in{gi}"
                )
                for gi, g in enumerate(hgroups)
            ]
            a2a_out = [
                dram.tile(
                    [NC * len(g) * HD, TSH], BF, tag=f"a2a_out{gi}", name=f"a2a_out{gi}"
                )
                for gi, g in enumerate(hgroups)
            ]
            pd_dram = [
                dram.tile([D, TSH], BF, tag=f"pd_dram{i}", name=f"pd_dram{i}")
                for i in range(2)
            ]


            with (
                tc.tile_pool(name="persist", bufs=1) as pp,
                tc.tile_pool(name="pt", bufs=6) as pt,
                tc.tile_pool(name="lt", bufs=2) as lt,
                tc.tile_pool(name="ot", bufs=8) as ot,
            ):
                ones_bf = pp.tile([KP, KP], BF, tag="ones")
                nc.vector.memset(ones_bf[:], 1.0)
                cos_sb = pp.tile([HD, N], F32, tag="cos")
                sin_sb = pp.tile([HD, N], F32, tag="sin")
                tri_sb = pp.tile([KP, KP], BF, tag="tri")

                # persistent activation tiles
                qT_sb = [
                    [
                        pp.tile([HD, N], BF, tag=f"qT_{b}_{h}", name=f"qT_{b}_{h}")
                        for h in range(HQ)
                    ]
                    for b in range(B)
                ]
                kT_sb = [
                    pp.tile([HD, N], BF, tag=f"kT_{b}", name=f"kT_{b}")
                    for b in range(B)
                ]
                v_sb = [
                    [
                        pp.tile([KP, HD], BF, tag=f"v_{b}_{kt}", name=f"v_{b}_{kt}")
                        for kt in range(NKT)
                    ]
                    for b in range(B)
                ]

                # ---- stage 1: qkv projection + rope ----------------------
                with (
                    tc.tile_pool(name="wpool", bufs=1) as wpool,
                    tc.tile_pool(name="xs", bufs=2) as xs,
                    tc.tile_pool(name="qc", bufs=2) as qcp,
                    tc.tile_pool(name="rt", bufs=2) as rt,
                    tc.tile_pool(name="ps1", bufs=1, space="PSUM") as ps1,
                ):
                    # wide weight loads: a few big rearranged DMAs per tensor
                    # (split so the first chunks land quickly for block 0)
                    wq_all = wpool.tile([KP, KC * SD], BF, tag="wq_all")
                    wk_all = wpool.tile([KP, KC * HD], BF, tag="wk_all")
                    wv_all = wpool.tile([KP, KC * HD], BF, tag="wv_all")
                    def _wpart(dst_all, srcT, fd, p0, pn):
                        nc.scalar.dma_start(
                            dst_all[
                                :, p0 * fd : (p0 + pn) * fd
                            ].rearrange("p (a c) -> p a c", a=pn),
                            srcT[
                                p0 * KP : (p0 + pn) * KP, :
                            ].rearrange("(a p) c -> p a c", p=KP),
                        )

                    # interleaved so the kc=0..7 slices of all three weights
                    # land first (the qkv matmuls consume them in kc order)
                    _wpart(wq_all, wqT, SD, 0, 4)
                    _wpart(wk_all, wkT, HD, 0, 8)
                    _wpart(wv_all, wvT, HD, 0, 8)
                    _wpart(wq_all, wqT, SD, 4, 8)
                    _wpart(wk_all, wkT, HD, 8, 12)
                    _wpart(wv_all, wvT, HD, 8, 12)
                    _wpart(wq_all, wqT, SD, 12, 10)
                    _wpart(wk_all, wkT, HD, 20, 12)
                    _wpart(wv_all, wvT, HD, 20, 12)
                    _wpart(wq_all, wqT, SD, 22, 10)
                    nc.scalar.dma_start(cos_sb[:], cosP[:])
                    nc.scalar.dma_start(sin_sb[:], sinP[:])
                    nc.scalar.dma_start(tri_sb[:], cmask[:])
                    for t in range(NTB):
                        b = t // NBB
                        n0 = (t % NBB) * TB  # position within batch
                        col0 = t * TB  # column in xT
                        qp = [
                            ps1.tile([KP, TB], F32, tag=f"qp{h}", name=f"qp{h}")
                            for h in range(HQ)
                        ]
                        kp = ps1.tile([KP, TB], F32, tag="kp", name="kp", bufs=2)
                        vp = ps1.tile([KP, TB], F32, tag="vp", name="vp", bufs=2)
                        for g in range(KC // NXB):
                            xt = xs.tile([KP, NXB * TB], BF, tag="xt")
                            nh = 2 if t == 0 and g == 0 else 1
                            hc = NXB // nh
                            for half in range(nh):
                                nc.sync.dma_start(
                                    xt[
                                        :, half * hc * TB : (half + 1) * hc * TB
                                    ].rearrange("p (a c) -> p a c", a=hc),
                                    xT[
                                        (g * NXB + half * hc) * KP : (
                                            g * NXB + (half + 1) * hc
                                        )
                                        * KP,
                                        col0 : col0 + TB,
                                    ].rearrange("(a p) c -> p a c", p=KP),
                                )
                            for j in range(NXB):
                                kc = g * NXB + j
                                xsl = xt[:, j * TB : (j + 1) * TB]
                                st = kc == 0
                                sp_ = kc == KC - 1
                                for h in range(HQ):
                                    nc.tensor.matmul(
                                        qp[h][:],
                                        wq_all[
                                            :, kc * SD + h * HD : kc * SD + (h + 1) * HD
                                        ],
                                        xsl,
                                        start=st,
                                        stop=sp_,
                                    )
                                nc.tensor.matmul(
                                    kp[:],
                                    wk_all[:, kc * HD : (kc + 1) * HD],
                                    xsl,
                                    start=st,
                                    stop=sp_,
                                )
                                nc.tensor.matmul(
                                    vp[:],
                                    wv_all[:, kc * HD : (kc + 1) * HD],
                                    xsl,
                                    start=st,
                                    stop=sp_,
                                )
                        # single fast ACT copy frees each PSUM bank; rope runs
                        # on DVE from SBUF without stalling the next block's
                        # matmuls
                        qk_c = []
                        for h in range(HQ):
                            c = qcp.tile([KP, TB], F32, tag=f"qc{h}", name=f"qc{h}")
                            nc.scalar.copy(c[:], qp[h][:])
                            qk_c.append(c)
                        vc = ot.tile([HD, TB], BF, tag="vc", bufs=3)
                        if t == NTB - 1:
                            nc.scalar.copy(vc[:], vp[:])
                            ksrc = qcp.tile([KP, TB], F32, tag="kc_")
                            nc.scalar.copy(ksrc[:], kp[:])
                        else:
                            nc.vector.tensor_copy(vc[:], vp[:])
                            ksrc = kp
                        # transpose this block's v tiles SBUF->SBUF right
                        # away (sync queue: long waits must not block the
                        # scalar FIFO, which carries the attention exps)
                        for s in range(TB // KP):
                            kt = (t % NBB) * (TB // KP) + s
                            nc.sync.dma_start_transpose(
                                v_sb[b][kt][:],
                                vc[:, s * KP : (s + 1) * KP],
                            )

                        cs_t = cos_sb[0:HH, n0 : n0 + TB]
                        cs_b = cos_sb[HH:HD, n0 : n0 + TB]
                        ss_t = sin_sb[0:HH, n0 : n0 + TB]
                        ss_b = sin_sb[HH:HD, n0 : n0 + TB]
                        for src, dst in [(qk_c[h], qT_sb[b][h]) for h in range(HQ)] + [
                            (ksrc, kT_sb[b])
                        ]:
                            t1 = rt.tile([HH, TB], F32, tag="t1")
                            t2 = rt.tile([HH, TB], F32, tag="t2")
                            nc.vector.tensor_tensor(t1[:], src[0:HH, :], cs_t, AX.mult)
                            nc.vector.tensor_tensor(t2[:], src[HH:HD, :], ss_b, AX.mult)
                            nc.vector.tensor_tensor(
                                dst[0:HH, n0 : n0 + TB], t1[:], t2[:], AX.subtract
                            )
                            t3 = rt.tile([HH, TB], F32, tag="t3")
                            t4 = rt.tile([HH, TB], F32, tag="t4")
                            nc.vector.tensor_tensor(t3[:], src[0:HH, :], ss_t, AX.mult)
                            nc.vector.tensor_tensor(t4[:], src[HH:HD, :], cs_b, AX.mult)
                            nc.vector.tensor_tensor(
                                dst[HH:HD, n0 : n0 + TB], t3[:], t4[:], AX.add
                            )

                # ---- stage 2: flash attention (no max subtraction) -------
                # stage-4 pools open early so wo-weight prefetch DMAs overlap
                # stage 2 and ride out the collectives
                NQB = N // TB
                DIAG = TB // KP
                LAG = 4
                with (
                    tc.tile_pool(name="s4", bufs=1) as p4,
                    tc.tile_pool(name="wos", bufs=4) as wos,
                    tc.tile_pool(name="fo", bufs=2) as fop,
                ):
                    NA0 = NC * len(hgroups[0])
                    wt_pre = {}
                    for mo in range(3):
                        wt = wos.tile([KP, NA0 * KP], BF, tag="wt", name=f"wtp{mo}")
                        nc.gpsimd.dma_start(wt[:], woL[mo][:, : NA0 * KP])
                        wt_pre[mo] = wt
                    ao_slice = {}  # kc -> AP into a stage-4 aog tile
                    kc_order = []
                    ps2_cm = tc.tile_pool(name="ps2", bufs=1, space="PSUM")
                    ps2 = ps2_cm.__enter__()
                    pending = [None]

                    def _fin_stage():
                        # advance one stage of the previous round's
                        # normalize/scatter. Staged across the next round's
                        # first steps so (a) the op-bank-freeing copies get
                        # into the ACT FIFO right after the first exp and
                        # (b) the Ln/exp chain interleaves between exps
                        # instead of delaying them all.
                        if pending[0] is None:
                            return
                        f_ops, f_lrs, f_grp, f_gi, f_b, f_qb, opc, rbcs, stg = (
                            pending[0]
                        )
                        f_ng = len(f_grp)
                        if stg == 0:
                            for i in range(f_ng):
                                c = lt.tile([HD, TB], F32, tag=f"opc{i}")
                                nc.scalar.copy(c[:], f_ops[i][:])
                                opc.append(c)
                        elif stg == 1:
                            for i in range(f_ng):
                                lnl = lt.tile([HD, TB], F32, tag="lnl")
                                nc.scalar.activation(
                                    lnl[:], f_lrs[i][:], AF.Ln
                                )
                                rbc = lt.tile([HD, TB], F32, tag="rbc")
                                nc.scalar.activation(
                                    rbc[:], lnl[:], AF.Exp, scale=-1.0
                                )
                                rbcs.append(rbc)
                        else:
                            for i, h in enumerate(f_grp):
                                outT = ot.tile([HD, TB], BF, tag="outT")
                                nc.vector.tensor_tensor(
                                    outT[:], opc[i][:], rbcs[i][:], AX.mult
                                )
                                # scatter to a2a_in: dest core j gets
                                # tokens [j*TSH, (j+1)*TSH)
                                g0 = (f_b * N + f_qb * TB) // TSH
                                npc = TB // TSH if TB >= TSH else 1
                                sdg = f_ng * HD
                                for jj in range(npc):
                                    j = g0 + jj
                                    r0 = j * sdg + i * HD
                                    nc.sync.dma_start(
                                        a2a_in[f_gi][r0 : r0 + HD, :],
                                        outT[:, jj * TSH : (jj + 1) * TSH],
                                    )
                            pending[0] = None
                            return
                        pending[0] = (
                            f_ops, f_lrs, f_grp, f_gi, f_b, f_qb,
                            opc, rbcs, stg + 1,
                        )

                    def _finalize():
                        while pending[0] is not None:
                            _fin_stage()

                    # round order: A-b0, then A-b1 interleaved with B-b0
                    # (gives the static scheduler ready PE work across the
                    # stage-1 -> batch-1 seam), then B-b1. Collectives still
                    # fire right after their group's last round.
                    round_order = [
                        (gi, b, qb)
                        for gi in range(len(hgroups))
                        for b in range(B)
                        for qb in range(NQB)
                    ]
                    emitted = {gi: 0 for gi in range(len(hgroups))}
                    per_group = B * NQB
                    for gi, b, qb in round_order:
                        grp = hgroups[gi]
                        ng = len(grp)
                        if True:
                            if True:
                                ops = [
                                    ps2.tile(
                                        [HD, TB], F32, tag=f"op{i}", name=f"op{i}",
                                        bufs=1,
                                    )
                                    for i in range(ng)
                                ]
                                lrs = [
                                    ps2.tile(
                                        [KP, TB], F32, tag=f"lr{i}", name=f"lr{i}",
                                        bufs=1,
                                    )
                                    for i in range(ng)
                                ]
                                nkt = (qb + 1) * DIAG
                                ptiles = {}
                                for step in range(nkt + LAG):
                                    if step < nkt:
                                        kt = step
                                        jd = kt - qb * DIAG
                                        c0 = jd * KP if jd > 0 else 0
                                        smega = ps2.tile(
                                            [KP, ng * TB], F32, tag="smega",
                                            name="smega", bufs=2,
                                        )
                                        # both heads share the kT stationary;
                                        # adjacent emission lets the LDW
                                        # dedupe pass drop the second load
                                        for i, h in enumerate(grp):
                                            nc.tensor.matmul(
                                                smega[:, i * TB + c0 : (i + 1) * TB],
                                                kT_sb[b][:, kt * KP : (kt + 1) * KP],
                                                qT_sb[b][h][
                                                    :, qb * TB + c0 : (qb + 1) * TB
                                                ],
                                                start=True,
                                                stop=True,
                                            )
                                        P = pt.tile([KP, ng * TB], BF, tag="P")
                                        # one wide exp across both heads'
                                        # tiles; unused diag cols hold stale
                                        # but finite scores (never read)
                                        nc.scalar.activation(
                                            P[:], smega[:], AF.Exp, scale=SCALE
                                        )
                                        if jd >= 0:
                                            for i in range(ng):
                                                nc.vector.tensor_tensor(
                                                    P[:, i * TB + c0 : i * TB + c0 + KP],
                                                    P[:, i * TB + c0 : i * TB + c0 + KP],
                                                    tri_sb[:],
                                                    AX.mult,
                                                )
                                        ptiles[kt] = (P, c0)
                                        if step <= 2:
                                            _fin_stage()
                                    if step >= LAG:
                                        kt = step - LAG
                                        P, c0 = ptiles[kt]
                                        st = kt == 0
                                        sp_ = kt == nkt - 1
                                        ndiag0 = qb * DIAG
                                        # denominator: off-diag kt tiles pair
                                        # up via one DVE add so the ones-
                                        # matmul streams half the columns
                                        if kt < ndiag0:
                                            if kt % 2 == 1:
                                                Pm, _ = ptiles[kt - 1]
                                                Pp = pt.tile(
                                                    [KP, ng * TB], BF,
                                                    tag="Pp", bufs=2,
                                                )
                                                nc.vector.tensor_tensor(
                                                    Pp[:], Pm[:], P[:], AX.add
                                                )
                                                for i in range(ng):
                                                    nc.tensor.matmul(
                                                        lrs[i][:],
                                                        ones_bf[:],
                                                        Pp[:, i * TB : (i + 1) * TB],
                                                        start=(kt == 1),
                                                        stop=False,
                                                    )
                                        else:
                                            for i in range(ng):
                                                nc.tensor.matmul(
                                                    lrs[i][:, c0:TB],
                                                    ones_bf[:],
                                                    P[:, i * TB + c0 : (i + 1) * TB],
                                                    start=(kt == 0),
                                                    stop=sp_,
                                                )
                                        for i in range(ng):
                                            nc.tensor.matmul(
                                                ops[i][:, c0:TB],
                                                v_sb[b][kt][:],
                                                P[:, i * TB + c0 : (i + 1) * TB],
                                                start=st,
                                                stop=sp_,
                                            )
                                pending[0] = (
                                    ops, lrs, grp, gi, b, qb, [], [], 0
                                )
                        emitted[gi] += 1
                        if emitted[gi] == per_group:
                            # group done: flush the tail + fire its a2a
                            _finalize()
                            nc.gpsimd.collective_compute(
                                "AllToAll",
                                AX.bypass,
                                replica_groups=[list(range(NC))],
                                ins=[a2a_in[gi].opt()],
                                outs=[a2a_out[gi].opt()],
                            )
                    for grp in hgroups:
                        for i in range(NC):
                            for hh, h in enumerate(grp):
                                kc_order.append(i * HQ + h)

                    ps2_cm.__exit__(None, None, None)
                    # ---- stage 4: output projection, N pipelined passes --
                    # pass i accumulates collective group i's kcs for ALL mo
                    # as soon as its a2a lands, merging the previous pass's
                    # partial (bf16 spill to DRAM, batched prefetch reload).
                    # woL columns are host-packed in kc_order, so pass i
                    # reads a contiguous column range.
                    pass_kcs = []
                    off = 0
                    for g in hgroups:
                        n = NC * len(g)
                        pass_kcs.append((off, kc_order[off : off + n]))
                        off += n
                    NPB = 8  # mo per partial-reload batch
                    with (
                        tc.tile_pool(name="ps4", bufs=2, space="PSUM") as ps4,
                    ):
                        for pi, (coff, kcs) in enumerate(pass_kcs):
                            last = pi == len(pass_kcs) - 1
                            nkc = len(kcs)
                            grp = hgroups[pi]
                            nblk = NC * len(grp)
                            aog = p4.tile(
                                [KP, nblk * TSH], BF, tag=f"aog{pi}",
                                name=f"aog{pi}",
                            )
                            nc.sync.dma_start(
                                aog[:].rearrange("p (n t) -> p n t", n=nblk),
                                a2a_out[pi][:, :].rearrange(
                                    "(n p) t -> p n t", p=KP
                                ),
                            )
                            for i in range(NC):
                                for hh, h in enumerate(grp):
                                    kc = i * HQ + h
                                    n = i * len(grp) + hh
                                    ao_slice[kc] = aog[
                                        :, n * TSH : (n + 1) * TSH
                                    ]
                            pb_batches = None
                            if pi > 0:
                                pd_src = pd_dram[(pi - 1) % 2]
                                pb_batches = []
                                for gb in range(MO // NPB):
                                    pb = fop.tile(
                                        [KP, NPB * TSH], BF, tag="pb", bufs=2,
                                        name=f"pb{pi}_{gb}",
                                    )
                                    nc.sync.dma_start(
                                        pb[:].rearrange("p (a t) -> p a t", a=NPB),
                                        pd_src[
                                            gb * NPB * KP : (gb + 1) * NPB * KP, :
                                        ].rearrange("(a p) t -> p a t", p=KP),
                                    )
                                    pb_batches.append(pb)
                            for mo in range(MO):
                                if pi == 0 and mo in wt_pre:
                                    wt = wt_pre.pop(mo)
                                else:
                                    wt = wos.tile(
                                        [KP, nkc * KP], BF,
                                        tag=("wt" if pi == 0 else "wtb"),
                                    )
                                    nc.gpsimd.dma_start(
                                        wt[:],
                                        woL[mo][:, coff * KP : (coff + nkc) * KP],
                                    )
                                fp = ps4.tile([KP, TSH], F32, tag="fp")
                                for idx, kc in enumerate(kcs):
                                    nc.tensor.matmul(
                                        fp[:],
                                        wt[:, idx * KP : (idx + 1) * KP],
                                        ao_slice[kc],
                                        start=(idx == 0),
                                        stop=(idx == nkc - 1),
                                    )
                                if pi == 0:
                                    pt_ = fop.tile([KP, TSH], BF, tag="pac", bufs=3)
                                    nc.scalar.copy(pt_[:], fp[:])
                                    nc.sync.dma_start(
                                        pd_dram[0][mo * KP : (mo + 1) * KP, :],
                                        pt_[:],
                                    )
                                else:
                                    pb_sl = pb_batches[mo // NPB][
                                        :, (mo % NPB) * TSH : (mo % NPB + 1) * TSH
                                    ]
                                    if last:
                                        fo = fop.tile(
                                            [KP, TSH], F32, tag="fo", bufs=3
                                        )
                                        nc.vector.tensor_tensor(
                                            fo[:], fp[:], pb_sl, AX.add
                                        )
                                        nc.sync.dma_start(
                                            finalT[mo * KP : (mo + 1) * KP, :],
                                            fo[:],
                                        )
                                    else:
                                        fo = fop.tile(
                                            [KP, TSH], BF, tag="fom", bufs=3
                                        )
                                        nc.vector.tensor_tensor(
                                            fo[:], fp[:], pb_sl, AX.add
                                        )
                                        nc.sync.dma_start(
                                            pd_dram[pi % 2][
                                                mo * KP : (mo + 1) * KP, :
                                            ],
                                            fo[:],
                                        )

    if DEDUPE_LDW:
        _dedupe_ldweights(nc, mybir)
    if COALESCE_UPDATES:
        _coalesce_pe_updates(nc, mybir)
    if split_waits:
        _split_wide_waits(nc, mybir)
    return nc


# --------------------------------------------------------------------------
def host_prep(x, wq, wk, wv, wo, cos, sin, B, N, D, NH, NKV):
    """Build the 8 per-core input maps."""
    HQ = NH // NC
    DQ = NH * HD
    TOK = B * N
    MO = D // KP

    perm = np.concatenate([np.arange(0, HD, 2), np.arange(1, HD, 2)])

    x2 = np.ascontiguousarray(x.reshape(TOK, D).T).astype(BF16)  # [D, TOK]
    cosT = np.ascontiguousarray(cos.T).astype(np.float32)  # [HD//2, N]
    sinT = np.ascontiguousarray(sin.T).astype(np.float32)
    cosP = np.concatenate([cosT, cosT], axis=0)  # duplicated halves [HD, N]
    sinP = np.concatenate([sinT, sinT], axis=0)

    # wo layout: woL[mo, p, kc*128+m] = wo[mo*128+m, kc*128+p], with the kc
    # axis packed in the device's collective-group order (pass A cols first)
    hgroups = _hgroups(HQ)
    kc_pack = [i * HQ + h for g in hgroups for i in range(NC) for h in g]
    wo4 = wo.reshape(MO, KP, DQ // KP, KP)  # [mo, m, kc, p]
    woL = wo4.transpose(0, 3, 2, 1)[:, :, kc_pack, :]
    woL = np.ascontiguousarray(woL.reshape(MO, KP, DQ)).astype(BF16)

    # single lower-triangle mask for the diagonal-band 128-col slice
    qt = np.arange(KP)[None, :]
    kt = np.arange(KP)[:, None]
    cmask = (qt >= kt).astype(np.float32).astype(BF16)

    in_maps = []
    for i in range(NC):
        wq_i = wq[i * HQ * HD : (i + 1) * HQ * HD]  # [HQ*HD, D]
        wq_i = wq_i.reshape(HQ, HD, D)[:, perm, :].reshape(HQ * HD, D)
        wqT = np.ascontiguousarray(wq_i.T).astype(BF16)
        wk_i = wk[i * HD : (i + 1) * HD][perm]
        wkT = np.ascontiguousarray(wk_i.T).astype(BF16)
        wv_i = wv[i * HD : (i + 1) * HD]
        wvT = np.ascontiguousarray(wv_i.T).astype(BF16)
        in_maps.append(
            {
                "xT": x2,
                "wqT": wqT,
                "wkT": wkT,
                "wvT": wvT,
                "woL": woL,
                "cosP": cosP,
                "sinP": sinP,
                "cmask": cmask,
            }
        )
    return in_maps


_NC_CACHE = {}


def _get_nc(B, N, D, NH, NKV):
    key = (B, N, D, NH, NKV)
    if key not in _NC_CACHE:
        _NC_CACHE[key] = build_attention_nc(B, N, D, NH, NKV)
    return _NC_CACHE[key]


def run(x, wq, wk, wv, wo, cos, sin, mask, start_pos, trace=False, **trace_kw):
    from concourse.bass_utils import run_bass_kernel_spmd

    x = np.asarray(x)
    B, N, D = x.shape
    NH = 32
    NKV = 8
    nc = _get_nc(B, N, D, NH, NKV)
    in_maps = host_prep(
        x,
        np.asarray(wq),
        np.asarray(wk),
        np.asarray(wv),
        np.asarray(wo),
        np.asarray(cos),
        np.asarray(sin),
        B,
        N,
        D,
        NH,
        NKV,
    )
    res = run_bass_kernel_spmd(nc, in_maps, list(range(NC)), trace=trace, **trace_kw)
    parts = [np.asarray(res.results[i]["finalT"], np.float32).T for i in range(NC)]
    out = np.concatenate(parts, axis=0)  # [TOK, D]
    return np.ascontiguousarray(out.reshape(B, N, D)), res


def kernel(x, wq, wk, wv, wo, cos, sin, mask, start_pos):
    out, _ = run(x, wq, wk, wv, wo, cos, sin, mask, start_pos)
    return out
